# revision 37
# baseline (speedup 1.0000x reference)
"""Segmented (ragged) single-query attention on 8 TRN2 NeuronCores.

Problem: B=32 batch rows, each with one query q[256], keys/values K/V[4096,256]
and 64 sorted separator positions. Segment i of row b covers key positions p
with seps[b,i] < p < seps[b,i+1]; softmax attention is computed independently
per segment. Output y[32,63,256] (+ y_mask of ones).

Sharding: data-parallel over B - each of the 8 cores handles 4 batch rows
(r = 0..3), no cross-core communication.

Layout: G=4 key positions per SBUF partition - position(p, c, g) =
c*512 + p*4 + g - so every DMA moves 4 KiB contiguous per partition
(4x fewer DMA packets; the packet rate, ~63 ns per 1 KiB packet per engine,
is what bounds the 33.5 MB/core stream otherwise). 8 chunks of 512 positions.

Per-core pipeline, software-skewed one chunk (stage B of chunk c-1 issues
after stage A of chunk c, so each strict-FIFO engine queue only holds work
whose inputs are in flight):
  stage A(c):
    K/V DMA           one dma_start per (row, chunk), rings split across the
                      sync and scalar HWDGE queues
    prefix = cumsum(K*qb) custom DVE op ANT_PREFIX_DOT (scan(ADD, Src0*Src1)),
                      one [128,1024] instruction per row; qb = q/16 broadcast
    e[:, r*4+g]       = prefix at column 256g+255 minus at 256(g-1)+255
    x = exp(e)        ScalarE (no max-subtraction: e ~ N(0,1), exp cannot
                      overflow; softmax weights are shift-invariant)
    mask_g[p, r*63+i] custom DVE op ANT_SEG_RANGE_MASK:
                      (lo[r,i] < pos(p,c,g)) & (hi[r,i] > pos(p,c,g))
  stage B(c):
    wx_g = mask_g * x[:, :, g]   GpSimd tensor_tensor (x broadcast over i),
                      written as float32r
    tot += wx_g       GpSimd running totals (for den)
    num[63,256] += wx_g[r].T @ V[r,g]   PE float32r matmul (even-N fast mode),
                      PSUM accumulation over all 32 (c, g)
  finale: den = tot.T @ ones (one matmul), y = num * 1/max(den, 1e-30);
  empty segments give num=0, den=0 -> y=0, matching the reference.
"""

import sys

for _p in ("/opt/trn_rl_repo",):
    if _p not in sys.path:
        sys.path.insert(0, _p)

import numpy as np

import concourse.bass as bass
import concourse.tile as tile
from concourse import bacc, mybir
from concourse.bass_utils import run_bass_kernel_spmd

def _register_range_mask_op():
    """Custom DVE op: out = (in0 < s0) & (in1 > s0) ? 1 : 0 — the segment
    membership mask (lo < pos < hi) in ONE DVE instruction instead of a
    tensor_scalar compare + scalar_tensor_tensor combine."""
    import numpy as _np

    from concourse import dve_ops as _dops
    from concourse.dve_spec import C0, One, Spec, Src0, Src1, Zero, lower, select
    from concourse.dve_uop import DveOpSpec

    name = "ANT_SEG_RANGE_MASK"
    for _op in _dops.OPS:
        if _op.name == name:
            return _op
    spec = Spec(
        body=select((Src0 < C0) & (Src1 > C0), One, Zero),
        reference=lambda in0, in1, c0, c1, c2: _np.where(
            (_np.asarray(in0, _np.float32) < c0) & (_np.asarray(in1, _np.float32) > c0),
            _np.float32(1.0),
            _np.float32(0.0),
        ).astype(_np.float32),
    )
    row = max(_dops._SUB_OPCODE_FOR_NAME.values()) + 1
    assert row < 0x20
    shas = {}
    for ver in ("v3", "v4"):
        shas[ver] = DveOpSpec(
            name=name, opcode=row, uops=lower(spec, ver=ver), rd1_en=True
        ).sha(ver)
    op = _dops.DveOp(name, spec, subdim=False, uops_sha=shas)
    _dops.OPS.append(op)
    _dops.CUSTOM_DVE_SPECS[name] = spec
    _dops._SUB_OPCODE_FOR_NAME[name] = row
    return op


RANGE_MASK_OP = _register_range_mask_op()


def _register_prefix_dot_op():
    """Custom DVE op: out[p, t] = sum_{u<=t} in0[p, u] * in1[p, u] — running
    dot-product along the free dim. One [128, G*256] instruction computes a
    whole K-row-group's q-dot prefix; the per-group sums are differences of
    the prefix at group boundaries."""
    import numpy as _np

    from concourse import dve_ops as _dops
    from concourse.dve_spec import AluOp, Spec, Src0, Src1, lower, scan
    from concourse.dve_uop import DveOpSpec

    name = "ANT_PREFIX_DOT"
    for _op in _dops.OPS:
        if _op.name == name:
            return _op

    def _ref(in0, in1, c0, c1, c2):
        a = _np.asarray(in0, _np.float32)
        b = _np.asarray(in1, _np.float32)
        prod = (a.reshape(b.shape) * b).reshape(a.shape[0], -1)
        return _np.cumsum(prod, axis=-1).astype(_np.float32).reshape(a.shape)

    spec = Spec(body=scan(AluOp.ADD, Src0 * Src1), reference=_ref)
    row = max(_dops._SUB_OPCODE_FOR_NAME.values()) + 1
    assert row < 0x20
    shas = {}
    for ver in ("v3", "v4"):
        shas[ver] = DveOpSpec(
            name=name, opcode=row, uops=lower(spec, ver=ver), rd1_en=True
        ).sha(ver)
    op = _dops.DveOp(name, spec, subdim=False, uops_sha=shas)
    _dops.OPS.append(op)
    _dops.CUSTOM_DVE_SPECS[name] = spec
    _dops._SUB_OPCODE_FOR_NAME[name] = row
    return op


PREFIX_DOT_OP = _register_prefix_dot_op()

N_CORES = 8
B, LK, D, NSEP = 32, 4096, 256, 64
S = NSEP - 1  # 63 segments
RPC = B // N_CORES  # rows per core = 4
G = 4  # positions per partition (4 KiB DMA packets)
NCHUNK = LK // (128 * G)  # 8 chunks of 512 positions
F32 = mybir.dt.float32
F32R = mybir.dt.float32r
BF16 = mybir.dt.bfloat16
I16 = mybir.dt.int16
I32 = mybir.dt.int32
OP = mybir.AluOpType
AF = mybir.ActivationFunctionType


def build_nc():
    nc = bacc.Bacc(
        "TRN2",
        target_bir_lowering=False,
        debug=False,
        enable_asserts=False,
        num_devices=N_CORES,
    )
    q_d = nc.dram_tensor("Q", [RPC, D], F32, kind="ExternalInput").ap()
    k_d = nc.dram_tensor("K", [RPC, LK, D], F32, kind="ExternalInput").ap()
    v_d = nc.dram_tensor("V", [RPC, LK, D], F32R, kind="ExternalInput").ap()
    # seps int64 bytes passed as int32 pairs (little-endian; values < 4096 so
    # the low word at even indices is the value).
    s_d = nc.dram_tensor("seps", [1, 2 * RPC * NSEP], I32, kind="ExternalInput").ap()
    y_d = nc.dram_tensor("y", [RPC, S, D], F32, kind="ExternalOutput").ap()

    with tile.TileContext(nc) as tc:
        build_tile_kernel(tc, q_d, k_d, v_d, s_d, y_d)

    nc.compile()
    return nc


def build_tile_kernel(tc, q_d, k_d, v_d, s_d, y_d):
    nc = tc.nc
    from contextlib import ExitStack

    with ExitStack() as ctx:
        const = ctx.enter_context(tc.tile_pool(name="const", bufs=1))

        # --- constants -------------------------------------------------
        ones_col = const.tile([128, 1], F32, tag="ones_col")
        nc.vector.memset(ones_col[:], 1.0)
        ones_row = const.tile([1, 128], F32, tag="ones_row")
        nc.vector.memset(ones_row[:], 1.0)
        scale_row = const.tile([1, 128], F32, tag="scale_row")
        nc.vector.memset(scale_row[:], 1.0 / 16.0)

        # position of (partition p, chunk c, slot g) = c*128*G + p*G + g
        pos_i = const.tile([128, NCHUNK * G], I32, tag="pos_i")
        nc.gpsimd.iota(
            pos_i[:], pattern=[[128 * G, NCHUNK], [1, G]], base=0, channel_multiplier=G
        )
        pos_f = const.tile([128, NCHUNK * G], F32, tag="pos_f")
        nc.vector.tensor_copy(pos_f[:], pos_i[:])

        # --- seps -> lo/hi rows, broadcast across partitions -----------
        seps_raw = const.tile([1, 2 * RPC * NSEP], I32, tag="seps_raw")
        nc.sync.dma_start(seps_raw[:], s_d[:])
        seps_f = const.tile([1, RPC * NSEP], F32, tag="seps_f")
        raw_pairs = seps_raw[:].rearrange("p (n two) -> p n two", two=2)
        nc.vector.tensor_copy(
            seps_f[:].rearrange("p (n one) -> p n one", one=1),
            raw_pairs[:, :, 0:1],
        )
        # packed [1, 504]: cols 0:252 = lo (seps[r, 0:63]), 252:504 = hi
        # (seps[r, 1:64]), r-major blocks of 63.
        packed = const.tile([1, 2 * RPC * S], F32, tag="packed")
        by_row = seps_f[:].rearrange("p (r i) -> p r i", i=NSEP)
        nc.vector.tensor_copy(
            packed[:, 0 : RPC * S].rearrange("p (r i) -> p r i", i=S),
            by_row[:, :, 0:S],
        )
        nc.vector.tensor_copy(
            packed[:, RPC * S : 2 * RPC * S].rearrange("p (r i) -> p r i", i=S),
            by_row[:, :, 1 : S + 1],
        )

        with tc.tile_pool(name="setup_ps", bufs=2, space="PSUM") as setup_ps:
            lohi_ps = setup_ps.tile([128, 2 * RPC * S], F32, tag="lohi_ps")
            nc.tensor.matmul(lohi_ps[:], ones_row[:], packed[:], start=True, stop=True)
            lohi = const.tile([128, 2 * RPC * S], F32, tag="lohi")
            nc.vector.tensor_copy(lohi[:], lohi_ps[:])
            lo_b = lohi[:, 0 : RPC * S]
            hi_b = lohi[:, RPC * S : 2 * RPC * S]

            # --- q broadcast to all 128 partitions, one tile per row ---
            qb = []
            for r in range(RPC):
                q_sb = const.tile([1, D], F32, tag=f"q_sb{r}", name=f"q_sb{r}")
                nc.sync.dma_start(q_sb[:], q_d[r : r + 1, :])
                q_ps = setup_ps.tile(
                    [128, D], F32, tag=f"q_ps{r % 2}", name=f"q_ps{r}"
                )
                nc.tensor.matmul(q_ps[:], scale_row[:], q_sb[:], start=True, stop=True)
                qb_r = const.tile([128, D], F32, tag=f"qb{r}", name=f"qb{r}")
                nc.vector.tensor_copy(qb_r[:], q_ps[:])
                qb.append(qb_r)

        ones2 = const.tile([128, 2], F32, tag="ones2")
        nc.vector.memset(ones2[:], 1.0)
        ones2r = const.tile([128, 2], F32R, tag="ones2r")
        nc.vector.tensor_copy(ones2r[:], ones2[:])

        # --- PE HAM warm-up: ~5us of back-to-back bf16 matmuls at t=0
        # (overlapped with the first K/V DMAs). fp32-HIGH matmuls do not
        # trip the HAM busy detector, so without this the whole kernel
        # runs at the cold 1.2 GHz PE clock; once warm, the continuous
        # fp32 stream keeps the idle detector from re-throttling.
        with tc.tile_pool(name="warm_ps", bufs=1, space="PSUM") as wps:
            wa = const.tile([128, 128], BF16, tag="warm_a")
            nc.vector.memset(wa[:], 0.0)
            wb = const.tile([128, 512], BF16, tag="warm_b")
            nc.vector.memset(wb[:], 0.0)
            wp_ps = wps.tile([128, 512], F32, tag="warm_ps")
            for i in range(24):
                nc.tensor.matmul(wp_ps[:], wa[:], wb[:], start=True, stop=True)

        # --- PSUM accumulators: per row num [63,256] + den [63,2] ------
        acc_pool = ctx.enter_context(tc.tile_pool(name="acc", bufs=1, space="PSUM"))
        acc = [
            acc_pool.tile([S, D], F32, tag=f"acc{r}", name=f"acc{r}")
            for r in range(RPC)
        ]
        dacc = [
            acc_pool.tile([S, 2], F32, tag=f"dacc{r}", name=f"dacc{r}")
            for r in range(RPC)
        ]

        kpool = ctx.enter_context(tc.tile_pool(name="kp", bufs=3))
        vpool = ctx.enter_context(tc.tile_pool(name="vp", bufs=4))
        spool = ctx.enter_context(tc.tile_pool(name="scratch", bufs=8))
        ppool = ctx.enter_context(tc.tile_pool(name="pp", bufs=2))
        epool = ctx.enter_context(tc.tile_pool(name="ep", bufs=6))
        wpool = ctx.enter_context(tc.tile_pool(name="wp", bufs=4))

        # --- main loop: 8 chunks of 128*G positions, software-pipelined:
        # stage A(c) = DMA + e/x computation; stage B(c) = mask*x + matmuls.
        # B(c-1) is issued after A(c) so each engine's program order only
        # contains work whose inputs are already in flight (avoids strict-
        # FIFO head-of-line blocking, esp. on GpSimd: prod(c) runs while
        # exp(c-1) finishes, then wx(c-1)).
        def dma_stage(c):
            kt = []
            vt = []
            for r in range(RPC):
                # position (p, g) <- DRAM row 128*G*c + p*G + g: each
                # partition's source is G rows = G KiB contiguous -> 4 KiB
                # DMA packets instead of 1 KiB.
                k_t = kpool.tile([128, G * D], F32, tag=f"k{r}", name=f"k{c}_{r}")
                src = k_d[r, 128 * G * c : 128 * G * (c + 1), :].rearrange(
                    "(p g) d -> p (g d)", g=G
                )
                k_eng = nc.sync if r < 2 else nc.scalar
                k_eng.dma_start(k_t[:], src)
                kt.append(k_t)

                v_t = vpool.tile([128, G * D], F32R, tag=f"v{r}", name=f"v{c}_{r}")
                vsrc = v_d[r, 128 * G * c : 128 * G * (c + 1), :].rearrange(
                    "(p g) d -> p (g d)", g=G
                )
                v_eng = nc.scalar if r < 2 else nc.sync
                v_eng.dma_start(v_t[:], vsrc)
                vt.append(v_t)
            return c, kt, vt

        def compute_a(st):
            c, kt, vt = st
            # e[p, r*G+g] = (K . q)/16 via one running-dot scan per K row
            # group (all four rows into one tile); per-slot sums =
            # differences of the prefix at the group-boundary columns, done
            # as ONE copy + ONE subtract for the whole chunk.
            e_t = epool.tile([128, RPC * G], F32, tag="e", name=f"e{c}")
            scn = ppool.tile([128, RPC * G * D], F32, tag="scan", name=f"scan{c}")
            for r in range(RPC):
                q_v = qb[r][:].rearrange("p (one d) -> p one d", one=1)
                k_v = kt[r][:].rearrange("p (g d) -> p g d", d=D)
                _, q_bc = bass.broadcast_tensor_aps(k_v, q_v)
                nc.vector._custom_dve(
                    PREFIX_DOT_OP,
                    out=scn[:, r * G * D : (r + 1) * G * D],
                    in0=kt[r][:],
                    in1=q_bc,
                )
            ends = scn[:].rearrange("p (rg d) -> p rg d", d=D)[:, :, D - 1 : D]
            raw = ends.rearrange("p (r g) one -> p r (g one)", g=G)
            e_v = e_t[:].rearrange("p (r g) -> p r g", g=G)
            nc.vector.tensor_copy(e_v[:, :, 0:1], raw[:, :, 0:1])
            nc.vector.tensor_tensor(
                e_v[:, :, 1:G], raw[:, :, 1:G], raw[:, :, 0 : G - 1],
                op=OP.subtract,
            )
            x_t = epool.tile([128, RPC * G], F32, tag="x", name=f"x{c}")
            nc.scalar.activation(x_t[:], e_t[:], AF.Exp)

            # masks don't depend on chunk data -- issue with stage A
            ws = []
            for g in range(G):
                pos_c = pos_f[:, c * G + g : c * G + g + 1]
                w_t = wpool.tile([128, RPC * S], I16, tag=f"w{g}", name=f"w{c}_{g}")
                nc.vector._custom_dve(
                    RANGE_MASK_OP, out=w_t[:], in0=lo_b, in1=hi_b, s0=pos_c
                )
                ws.append(w_t)
            return c, vt, x_t, ws

        def stage_b(state):
            c, vt, x_t, ws = state
            wx = []
            for g in range(G):
                wx_t = wpool.tile(
                    [128, RPC * S], F32R, tag=f"wx{g}", name=f"wx{c}_{g}"
                )
                w_v = ws[g][:].rearrange("p (r i) -> p r i", i=S)
                # x column for (r, g): stride G over r at offset g
                x_v = x_t[:].rearrange("p (r g) -> p r g", g=G)[:, :, g : g + 1]
                w_bc, x_bc = bass.broadcast_tensor_aps(w_v, x_v)
                nc.gpsimd.tensor_tensor(
                    wx_t[:].rearrange("p (r i) -> p r i", i=S), w_bc, x_bc, op=OP.mult
                )
                wx.append(wx_t)
            first = c == 0
            last = c == NCHUNK - 1
            for r in range(RPC):
                for g in range(G):
                    lhs = wx[g][:, r * S : (r + 1) * S]
                    nc.tensor.matmul(
                        acc[r][:],
                        lhs,
                        vt[r][:, g * D : (g + 1) * D],
                        start=first and g == 0,
                        stop=last and g == G - 1,
                    )
                    # den rides the PE's idle time, reusing the same weights
                    nc.tensor.matmul(
                        dacc[r][:],
                        lhs,
                        ones2r[:],
                        start=first and g == 0,
                        stop=last and g == G - 1,
                    )

        # three-stage software pipeline: DMA(c) issues a full chunk ahead
        # of the compute that consumes it (so the DVE scans never wait on
        # K arrival), and the mask*x/matmul stage runs one further chunk
        # behind so the DVE chain (scan -> assembly -> exp -> wx) has two
        # chunk-times of slack before the PE needs its operands.
        pend_a = []
        pend_b = []
        for c in range(NCHUNK):
            pend_a.append(dma_stage(c))
            if len(pend_a) > 1:
                pend_b.append(compute_a(pend_a.pop(0)))
            if len(pend_b) > 1:
                stage_b(pend_b.pop(0))
        while pend_a:
            pend_b.append(compute_a(pend_a.pop(0)))
            stage_b(pend_b.pop(0))
        while pend_b:
            stage_b(pend_b.pop(0))

        # --- finalize: den = sum_p(tot01+tot23), y = num/max(den,1e-30) -
        fpool = ctx.enter_context(tc.tile_pool(name="fin", bufs=2))
        ypool = ctx.enter_context(tc.tile_pool(name="yout", bufs=2))
        for r in range(RPC):
            den_c = fpool.tile([S, 1], F32, tag="den", name=f"den{r}")
            nc.vector.tensor_scalar(
                den_c[:], dacc[r][:, 0:1], 1e-30, None, op0=OP.max
            )
            rec = fpool.tile([S, 1], F32, tag="rec", name=f"rec{r}")
            nc.vector.reciprocal(rec[:], den_c[:])
            y_sb = ypool.tile([S, D], F32, tag="y", name=f"y{r}")
            nc.vector.tensor_scalar(y_sb[:], acc[r][:, 0:D], rec[:], None, op0=OP.mult)
            nc.sync.dma_start(y_d[r, :, :], y_sb[:])


_NC_CACHE = None


def _get_nc():
    global _NC_CACHE
    if _NC_CACHE is None:
        _NC_CACHE = build_nc()
    return _NC_CACHE


def make_in_maps(Q, K, V, seps):
    Q = np.ascontiguousarray(np.asarray(Q, dtype=np.float32)).reshape(B, D)
    K = np.ascontiguousarray(np.asarray(K, dtype=np.float32))
    V = np.ascontiguousarray(np.asarray(V, dtype=np.float32))
    seps = np.ascontiguousarray(np.asarray(seps, dtype=np.int64))
    in_maps = []
    for i in range(N_CORES):
        sl = slice(i * RPC, (i + 1) * RPC)
        in_maps.append(
            {
                "Q": np.ascontiguousarray(Q[sl]),
                "K": np.ascontiguousarray(K[sl]),
                "V": np.ascontiguousarray(V[sl]),
                "seps": np.ascontiguousarray(seps[sl]).view(np.int32).reshape(1, -1),
            }
        )
    return in_maps


def kernel(Q, K, V, seps):
    nc = _get_nc()
    in_maps = make_in_maps(Q, K, V, seps)
    res = run_bass_kernel_spmd(nc, in_maps, core_ids=list(range(N_CORES)))
    y = np.concatenate([res.results[i]["y"] for i in range(N_CORES)], axis=0)
    y_mask = np.ones((B, S), dtype=np.float32)
    return (y, y_mask)


# revision 38
# speedup vs baseline: 1.1285x; 1.1285x over previous
"""Segmented (ragged) single-query attention on 8 TRN2 NeuronCores.

Problem: B=32 batch rows, each with one query q[256], keys/values K/V[4096,256]
and 64 sorted separator positions. Segment i of row b covers key positions p
with seps[b,i] < p < seps[b,i+1]; softmax attention is computed independently
per segment. Output y[32,63,256] (+ y_mask of ones).

Sharding: data-parallel over B - each of the 8 cores handles 4 batch rows
(r = 0..3), no cross-core communication.

Layout: G=4 key positions per SBUF partition - position(p, c, g) =
c*512 + p*4 + g - so every DMA moves 4 KiB contiguous per partition
(4x fewer DMA packets; the packet rate, ~63 ns per 1 KiB packet per engine,
is what bounds the 33.5 MB/core stream otherwise). 8 chunks of 512 positions.

Per-core pipeline, software-skewed one chunk (stage B of chunk c-1 issues
after stage A of chunk c, so each strict-FIFO engine queue only holds work
whose inputs are in flight):
  stage A(c):
    K/V DMA           one dma_start per (row, chunk), rings split across the
                      sync and scalar HWDGE queues
    prefix = cumsum(K*qb) custom DVE op ANT_PREFIX_DOT (scan(ADD, Src0*Src1)),
                      one [128,1024] instruction per row; qb = q/16 broadcast
    e[:, r*4+g]       = prefix at column 256g+255 minus at 256(g-1)+255
    x = exp(e)        ScalarE (no max-subtraction: e ~ N(0,1), exp cannot
                      overflow; softmax weights are shift-invariant)
    mask_g[p, r*63+i] custom DVE op ANT_SEG_RANGE_MASK:
                      (lo[r,i] < pos(p,c,g)) & (hi[r,i] > pos(p,c,g))
  stage B(c):
    wx_g = mask_g * x[:, :, g]   GpSimd tensor_tensor (x broadcast over i),
                      written as float32r
    tot += wx_g       GpSimd running totals (for den)
    num[63,256] += wx_g[r].T @ V[r,g]   PE float32r matmul (even-N fast mode),
                      PSUM accumulation over all 32 (c, g)
  finale: den = tot.T @ ones (one matmul), y = num * 1/max(den, 1e-30);
  empty segments give num=0, den=0 -> y=0, matching the reference.
"""

import sys

for _p in ("/opt/trn_rl_repo",):
    if _p not in sys.path:
        sys.path.insert(0, _p)

import numpy as np

import concourse.bass as bass
import concourse.tile as tile
from concourse import bacc, mybir
from concourse.bass_utils import run_bass_kernel_spmd

def _register_range_mask_op():
    """Custom DVE op: out = (in0 < s0) & (in1 > s0) ? 1 : 0 — the segment
    membership mask (lo < pos < hi) in ONE DVE instruction instead of a
    tensor_scalar compare + scalar_tensor_tensor combine."""
    import numpy as _np

    from concourse import dve_ops as _dops
    from concourse.dve_spec import C0, One, Spec, Src0, Src1, Zero, lower, select
    from concourse.dve_uop import DveOpSpec

    name = "ANT_SEG_RANGE_MASK"
    for _op in _dops.OPS:
        if _op.name == name:
            return _op
    spec = Spec(
        body=select((Src0 < C0) & (Src1 > C0), One, Zero),
        reference=lambda in0, in1, c0, c1, c2: _np.where(
            (_np.asarray(in0, _np.float32) < c0) & (_np.asarray(in1, _np.float32) > c0),
            _np.float32(1.0),
            _np.float32(0.0),
        ).astype(_np.float32),
    )
    row = max(_dops._SUB_OPCODE_FOR_NAME.values()) + 1
    assert row < 0x20
    shas = {}
    for ver in ("v3", "v4"):
        shas[ver] = DveOpSpec(
            name=name, opcode=row, uops=lower(spec, ver=ver), rd1_en=True
        ).sha(ver)
    op = _dops.DveOp(name, spec, subdim=False, uops_sha=shas)
    _dops.OPS.append(op)
    _dops.CUSTOM_DVE_SPECS[name] = spec
    _dops._SUB_OPCODE_FOR_NAME[name] = row
    return op


RANGE_MASK_OP = _register_range_mask_op()


def _register_prefix_dot_op():
    """Custom DVE op: out[p, t] = sum_{u<=t} in0[p, u] * in1[p, u] — running
    dot-product along the free dim. One [128, G*256] instruction computes a
    whole K-row-group's q-dot prefix; the per-group sums are differences of
    the prefix at group boundaries."""
    import numpy as _np

    from concourse import dve_ops as _dops
    from concourse.dve_spec import AluOp, Spec, Src0, Src1, lower, scan
    from concourse.dve_uop import DveOpSpec

    name = "ANT_PREFIX_DOT"
    for _op in _dops.OPS:
        if _op.name == name:
            return _op

    def _ref(in0, in1, c0, c1, c2):
        a = _np.asarray(in0, _np.float32)
        b = _np.asarray(in1, _np.float32)
        prod = (a.reshape(b.shape) * b).reshape(a.shape[0], -1)
        return _np.cumsum(prod, axis=-1).astype(_np.float32).reshape(a.shape)

    spec = Spec(body=scan(AluOp.ADD, Src0 * Src1), reference=_ref)
    row = max(_dops._SUB_OPCODE_FOR_NAME.values()) + 1
    assert row < 0x20
    shas = {}
    for ver in ("v3", "v4"):
        shas[ver] = DveOpSpec(
            name=name, opcode=row, uops=lower(spec, ver=ver), rd1_en=True
        ).sha(ver)
    op = _dops.DveOp(name, spec, subdim=False, uops_sha=shas)
    _dops.OPS.append(op)
    _dops.CUSTOM_DVE_SPECS[name] = spec
    _dops._SUB_OPCODE_FOR_NAME[name] = row
    return op


PREFIX_DOT_OP = _register_prefix_dot_op()

N_CORES = 8
B, LK, D, NSEP = 32, 4096, 256, 64
S = NSEP - 1  # 63 segments
RPC = B // N_CORES  # rows per core = 4
G = 4  # positions per partition (4 KiB DMA packets)
NCHUNK = LK // (128 * G)  # 8 chunks of 512 positions
F32 = mybir.dt.float32
F32R = mybir.dt.float32r
BF16 = mybir.dt.bfloat16
I16 = mybir.dt.int16
I32 = mybir.dt.int32
OP = mybir.AluOpType
AF = mybir.ActivationFunctionType


def build_nc():
    nc = bacc.Bacc(
        "TRN2",
        target_bir_lowering=False,
        debug=False,
        enable_asserts=False,
        num_devices=N_CORES,
    )
    q_d = nc.dram_tensor("Q", [RPC, D], F32, kind="ExternalInput").ap()
    k_d = nc.dram_tensor("K", [RPC, LK, D], F32, kind="ExternalInput").ap()
    v_d = nc.dram_tensor("V", [RPC, LK, D], F32R, kind="ExternalInput").ap()
    # seps int64 bytes passed as int32 pairs (little-endian; values < 4096 so
    # the low word at even indices is the value).
    s_d = nc.dram_tensor("seps", [1, 2 * RPC * NSEP], I32, kind="ExternalInput").ap()
    y_d = nc.dram_tensor("y", [RPC, S, D], F32, kind="ExternalOutput").ap()

    with tile.TileContext(nc) as tc:
        build_tile_kernel(tc, q_d, k_d, v_d, s_d, y_d)

    nc.compile()
    return nc


def build_tile_kernel(tc, q_d, k_d, v_d, s_d, y_d):
    nc = tc.nc
    from contextlib import ExitStack

    with ExitStack() as ctx:
        const = ctx.enter_context(tc.tile_pool(name="const", bufs=1))

        # --- constants -------------------------------------------------
        ones_col = const.tile([128, 1], F32, tag="ones_col")
        nc.vector.memset(ones_col[:], 1.0)
        ones_row = const.tile([1, 128], F32, tag="ones_row")
        nc.vector.memset(ones_row[:], 1.0)
        scale_row = const.tile([1, 128], F32, tag="scale_row")
        nc.vector.memset(scale_row[:], 1.0 / 16.0)

        # position of (partition p, chunk c, slot g) = c*128*G + p*G + g
        pos_i = const.tile([128, NCHUNK * G], I32, tag="pos_i")
        nc.gpsimd.iota(
            pos_i[:], pattern=[[128 * G, NCHUNK], [1, G]], base=0, channel_multiplier=G
        )
        pos_f = const.tile([128, NCHUNK * G], F32, tag="pos_f")
        nc.vector.tensor_copy(pos_f[:], pos_i[:])

        # --- seps -> lo/hi rows, broadcast across partitions -----------
        seps_raw = const.tile([1, 2 * RPC * NSEP], I32, tag="seps_raw")
        nc.sync.dma_start(seps_raw[:], s_d[:])
        seps_f = const.tile([1, RPC * NSEP], F32, tag="seps_f")
        raw_pairs = seps_raw[:].rearrange("p (n two) -> p n two", two=2)
        nc.vector.tensor_copy(
            seps_f[:].rearrange("p (n one) -> p n one", one=1),
            raw_pairs[:, :, 0:1],
        )
        # packed [1, 504]: cols 0:252 = lo (seps[r, 0:63]), 252:504 = hi
        # (seps[r, 1:64]), r-major blocks of 63.
        packed = const.tile([1, 2 * RPC * S], F32, tag="packed")
        by_row = seps_f[:].rearrange("p (r i) -> p r i", i=NSEP)
        nc.vector.tensor_copy(
            packed[:, 0 : RPC * S].rearrange("p (r i) -> p r i", i=S),
            by_row[:, :, 0:S],
        )
        nc.vector.tensor_copy(
            packed[:, RPC * S : 2 * RPC * S].rearrange("p (r i) -> p r i", i=S),
            by_row[:, :, 1 : S + 1],
        )

        with tc.tile_pool(name="setup_ps", bufs=2, space="PSUM") as setup_ps:
            lohi_ps = setup_ps.tile([128, 2 * RPC * S], F32, tag="lohi_ps")
            nc.tensor.matmul(lohi_ps[:], ones_row[:], packed[:], start=True, stop=True)
            lohi = const.tile([128, 2 * RPC * S], F32, tag="lohi")
            nc.vector.tensor_copy(lohi[:], lohi_ps[:])
            lo_b = lohi[:, 0 : RPC * S]
            hi_b = lohi[:, RPC * S : 2 * RPC * S]

            # --- q broadcast to all 128 partitions, one tile per row ---
            qb = []
            for r in range(RPC):
                q_sb = const.tile([1, D], F32, tag=f"q_sb{r}", name=f"q_sb{r}")
                nc.sync.dma_start(q_sb[:], q_d[r : r + 1, :])
                q_ps = setup_ps.tile(
                    [128, D], F32, tag=f"q_ps{r % 2}", name=f"q_ps{r}"
                )
                nc.tensor.matmul(q_ps[:], scale_row[:], q_sb[:], start=True, stop=True)
                qb_r = const.tile([128, D], F32, tag=f"qb{r}", name=f"qb{r}")
                nc.vector.tensor_copy(qb_r[:], q_ps[:])
                qb.append(qb_r)

        ones2 = const.tile([128, 2], F32, tag="ones2")
        nc.vector.memset(ones2[:], 1.0)
        ones2r = const.tile([128, 2], F32R, tag="ones2r")
        nc.vector.tensor_copy(ones2r[:], ones2[:])

        # --- PE HAM warm-up: ~5us of back-to-back bf16 matmuls at t=0
        # (overlapped with the first K/V DMAs). fp32-HIGH matmuls do not
        # trip the HAM busy detector, so without this the whole kernel
        # runs at the cold 1.2 GHz PE clock; once warm, the continuous
        # fp32 stream keeps the idle detector from re-throttling.
        with tc.tile_pool(name="warm_ps", bufs=1, space="PSUM") as wps:
            wa = const.tile([128, 128], BF16, tag="warm_a")
            nc.vector.memset(wa[:], 0.0)
            wb = const.tile([128, 512], BF16, tag="warm_b")
            nc.vector.memset(wb[:], 0.0)
            wp_ps = wps.tile([128, 512], F32, tag="warm_ps")
            for i in range(24):
                nc.tensor.matmul(wp_ps[:], wa[:], wb[:], start=True, stop=True)

        # --- PSUM accumulators: per row num [63,256] + den [63,2] ------
        acc_pool = ctx.enter_context(tc.tile_pool(name="acc", bufs=1, space="PSUM"))
        acc = [
            acc_pool.tile([S, D], F32, tag=f"acc{r}", name=f"acc{r}")
            for r in range(RPC)
        ]
        dacc = [
            acc_pool.tile([S, 2], F32, tag=f"dacc{r}", name=f"dacc{r}")
            for r in range(RPC)
        ]

        kpool = ctx.enter_context(tc.tile_pool(name="kp", bufs=4))
        vpool = ctx.enter_context(tc.tile_pool(name="vp", bufs=4))
        spool = ctx.enter_context(tc.tile_pool(name="scratch", bufs=8))
        ppool = ctx.enter_context(tc.tile_pool(name="pp", bufs=2))
        epool = ctx.enter_context(tc.tile_pool(name="ep", bufs=6))
        wpool = ctx.enter_context(tc.tile_pool(name="wp", bufs=4))

        # --- main loop: 8 chunks of 128*G positions, software-pipelined:
        # stage A(c) = DMA + e/x computation; stage B(c) = mask*x + matmuls.
        # B(c-1) is issued after A(c) so each engine's program order only
        # contains work whose inputs are already in flight (avoids strict-
        # FIFO head-of-line blocking, esp. on GpSimd: prod(c) runs while
        # exp(c-1) finishes, then wx(c-1)).
        def dma_stage(c):
            kt = []
            vt = []
            for r in range(RPC):
                # position (p, g) <- DRAM row 128*G*c + p*G + g: each
                # partition's source is G rows = G KiB contiguous -> 4 KiB
                # DMA packets instead of 1 KiB.
                k_t = kpool.tile([128, G * D], F32, tag=f"k{r}", name=f"k{c}_{r}")
                src = k_d[r, 128 * G * c : 128 * G * (c + 1), :].rearrange(
                    "(p g) d -> p (g d)", g=G
                )
                k_eng = nc.sync if r < 2 else nc.scalar
                k_eng.dma_start(k_t[:], src)
                kt.append(k_t)

                v_t = vpool.tile([128, G * D], F32R, tag=f"v{r}", name=f"v{c}_{r}")
                vsrc = v_d[r, 128 * G * c : 128 * G * (c + 1), :].rearrange(
                    "(p g) d -> p (g d)", g=G
                )
                v_eng = nc.scalar if r < 2 else nc.sync
                v_eng.dma_start(v_t[:], vsrc)
                vt.append(v_t)
            return c, kt, vt

        def compute_a(st):
            c, kt, vt = st
            # e[p, r*G+g] = (K . q)/16 via one running-dot scan per K row
            # group (all four rows into one tile); per-slot sums =
            # differences of the prefix at the group-boundary columns, done
            # as ONE copy + ONE subtract for the whole chunk.
            e_t = epool.tile([128, RPC * G], F32, tag="e", name=f"e{c}")
            scn = ppool.tile([128, RPC * G * D], F32, tag="scan", name=f"scan{c}")
            for r in range(RPC):
                q_v = qb[r][:].rearrange("p (one d) -> p one d", one=1)
                k_v = kt[r][:].rearrange("p (g d) -> p g d", d=D)
                _, q_bc = bass.broadcast_tensor_aps(k_v, q_v)
                nc.vector._custom_dve(
                    PREFIX_DOT_OP,
                    out=scn[:, r * G * D : (r + 1) * G * D],
                    in0=kt[r][:],
                    in1=q_bc,
                )
            ends = scn[:].rearrange("p (rg d) -> p rg d", d=D)[:, :, D - 1 : D]
            raw = ends.rearrange("p (r g) one -> p r (g one)", g=G)
            e_v = e_t[:].rearrange("p (r g) -> p r g", g=G)
            nc.vector.tensor_copy(e_v[:, :, 0:1], raw[:, :, 0:1])
            nc.vector.tensor_tensor(
                e_v[:, :, 1:G], raw[:, :, 1:G], raw[:, :, 0 : G - 1],
                op=OP.subtract,
            )
            x_t = epool.tile([128, RPC * G], F32, tag="x", name=f"x{c}")
            nc.scalar.activation(x_t[:], e_t[:], AF.Exp)

            # masks don't depend on chunk data -- issue with stage A
            ws = []
            for g in range(G):
                pos_c = pos_f[:, c * G + g : c * G + g + 1]
                w_t = wpool.tile([128, RPC * S], I16, tag=f"w{g}", name=f"w{c}_{g}")
                nc.vector._custom_dve(
                    RANGE_MASK_OP, out=w_t[:], in0=lo_b, in1=hi_b, s0=pos_c
                )
                ws.append(w_t)
            return c, vt, x_t, ws

        def stage_b(state):
            c, vt, x_t, ws = state
            wx = []
            for g in range(G):
                wx_t = wpool.tile(
                    [128, RPC * S], F32R, tag=f"wx{g}", name=f"wx{c}_{g}"
                )
                w_v = ws[g][:].rearrange("p (r i) -> p r i", i=S)
                # x column for (r, g): stride G over r at offset g
                x_v = x_t[:].rearrange("p (r g) -> p r g", g=G)[:, :, g : g + 1]
                w_bc, x_bc = bass.broadcast_tensor_aps(w_v, x_v)
                nc.gpsimd.tensor_tensor(
                    wx_t[:].rearrange("p (r i) -> p r i", i=S), w_bc, x_bc, op=OP.mult
                )
                wx.append(wx_t)
            first = c == 0
            last = c == NCHUNK - 1
            for r in range(RPC):
                for g in range(G):
                    lhs = wx[g][:, r * S : (r + 1) * S]
                    nc.tensor.matmul(
                        acc[r][:],
                        lhs,
                        vt[r][:, g * D : (g + 1) * D],
                        start=first and g == 0,
                        stop=last and g == G - 1,
                    )
                    # den rides the PE's idle time, reusing the same weights
                    nc.tensor.matmul(
                        dacc[r][:],
                        lhs,
                        ones2r[:],
                        start=first and g == 0,
                        stop=last and g == G - 1,
                    )

        # three-stage software pipeline: DMA(c) issues a full chunk ahead
        # of the compute that consumes it (so the DVE scans never wait on
        # K arrival), and the mask*x/matmul stage runs one further chunk
        # behind so the DVE chain (scan -> assembly -> exp -> wx) has two
        # chunk-times of slack before the PE needs its operands.
        pend_a = []
        pend_b = []
        for c in range(NCHUNK):
            pend_a.append(dma_stage(c))
            if len(pend_a) > 1:
                pend_b.append(compute_a(pend_a.pop(0)))
            if len(pend_b) > 1:
                stage_b(pend_b.pop(0))
        while pend_a:
            pend_b.append(compute_a(pend_a.pop(0)))
            stage_b(pend_b.pop(0))
        while pend_b:
            stage_b(pend_b.pop(0))

        # --- finalize: den = sum_p(tot01+tot23), y = num/max(den,1e-30) -
        fpool = ctx.enter_context(tc.tile_pool(name="fin", bufs=2))
        ypool = ctx.enter_context(tc.tile_pool(name="yout", bufs=2))
        for r in range(RPC):
            den_c = fpool.tile([S, 1], F32, tag="den", name=f"den{r}")
            nc.vector.tensor_scalar(
                den_c[:], dacc[r][:, 0:1], 1e-30, None, op0=OP.max
            )
            rec = fpool.tile([S, 1], F32, tag="rec", name=f"rec{r}")
            nc.vector.reciprocal(rec[:], den_c[:])
            y_sb = ypool.tile([S, D], F32, tag="y", name=f"y{r}")
            nc.vector.tensor_scalar(y_sb[:], acc[r][:, 0:D], rec[:], None, op0=OP.mult)
            nc.sync.dma_start(y_d[r, :, :], y_sb[:])


_NC_CACHE = None


def _get_nc():
    global _NC_CACHE
    if _NC_CACHE is None:
        _NC_CACHE = build_nc()
    return _NC_CACHE


def make_in_maps(Q, K, V, seps):
    Q = np.ascontiguousarray(np.asarray(Q, dtype=np.float32)).reshape(B, D)
    K = np.ascontiguousarray(np.asarray(K, dtype=np.float32))
    V = np.ascontiguousarray(np.asarray(V, dtype=np.float32))
    seps = np.ascontiguousarray(np.asarray(seps, dtype=np.int64))
    in_maps = []
    for i in range(N_CORES):
        sl = slice(i * RPC, (i + 1) * RPC)
        in_maps.append(
            {
                "Q": np.ascontiguousarray(Q[sl]),
                "K": np.ascontiguousarray(K[sl]),
                "V": np.ascontiguousarray(V[sl]),
                "seps": np.ascontiguousarray(seps[sl]).view(np.int32).reshape(1, -1),
            }
        )
    return in_maps


def kernel(Q, K, V, seps):
    nc = _get_nc()
    in_maps = make_in_maps(Q, K, V, seps)
    res = run_bass_kernel_spmd(nc, in_maps, core_ids=list(range(N_CORES)))
    y = np.concatenate([res.results[i]["y"] for i in range(N_CORES)], axis=0)
    y_mask = np.ones((B, S), dtype=np.float32)
    return (y, y_mask)
